# revision 6
# baseline (speedup 1.0000x reference)
"""BrickTube kernel for 8x Trainium2 NeuronCores.

The reference "BrickTube" module applies 80 tiny (2,2,2,2) gate cores to a
[B, 1024] state tensor. Every gate application is linear in x and
INPUT_DIM == BINDIM == OUTPUT_DIM == 1024, so the whole module collapses to

    out = x @ W,   W[i, :] = circuit(e_i)  (1024 x 1024)

W is built exactly on the host in float64 from `cores`, then the device runs a
batch-sharded dense matmul: each of the 8 cores computes y_c^T = W^T @ x_c^T
for its 4096-row shard of x.

Mixed-precision split-K: W's row norms span ~200x. The 768 contraction rows
with the smallest ||W_row|| (holding ~15% of the output energy) are computed
in fp8 e4m3 with MatmulPerfMode.DoubleRow (two 128-row k-subtiles per
instruction at 2x the fp16 MAC rate); the 256 largest rows stay fp16.
Host-measured rel err of this split vs float64: ~1.5e-2 (gate: 2e-2), and the
HW result matches the host simulation to 6 digits.

Scale folding: W8 = e4m3(W_sel8 * S), W16 = fp16(W_sel16 * S) with S a power
of two chosen so W8's absmax sits just under e4m3's 240 max-normal. All
matmuls then accumulate S*y into a single PSUM bank per output block, and the
PSUM->SBUF drain applies the exact 1/S scale for free (scaled copy). Output
is written fp16 (negligible extra error) and upcast on host.

All DRAM tensors are pre-arranged on the host into per-partition-contiguous
[128, ...] layouts so every DMA is a plain 2D block copy with 1-6KB
contiguous elements per partition (full HBM rate, cheap descriptor gen); no
rearranges on the device side.

Device kernel structure (per core):
  - 5 x 512-col warmup matmuls on zeros cover the PE HAM clock ramp while the
    first x8/w8 pieces are in flight (x8 j=0 split on the idle Sync ring,
    weights + remaining x chunks on the Scalar ring).
  - chunk j=0: k-pair-outer fp8 phase then fp16 phase, so the first w8/x8
    pieces cover the first 8 matmuls and w16's later arrival can't stall.
  - chunks j>=1: m-outer "full-finish" — each PSUM bank's 5 matmuls (3 fp8
    DoubleRow + 2 fp16), an immediate scaled drain (DVE/ACT alternating by m
    parity) and a pair-DMA out, so the output flush rides along the chunk's
    compute and the kernel tail only waits on the last 128KB.
"""

import math

import ml_dtypes
import numpy as np

# ---- problem constants (hardcoded per contract) ----
B = 32768
D = 1024
N_CORES = 8
NPC = B // N_CORES  # 4096 batch rows per core

BOND = 2
Q = 10
N_LAYERS = 8
PAIRS1 = [(i, i + 1) for i in range(0, Q, 2)]
PAIRS2 = [(i, (i + 1) % Q) for i in range(1, Q, 2)]
HALF = Q // 2

K8 = 768  # contraction rows computed in fp8 (smallest-norm rows of W)
K16 = D - K8  # rows kept in fp16
T8 = K8 // 128  # 6 fp8 k-subtiles -> 3 DoubleRow pairs
T16 = K16 // 128  # 2 fp16 k-subtiles
JC = NPC // 512  # 8 batch column chunks
MC = D // 128  # 8 output-row chunks


def build_w(cores: np.ndarray) -> np.ndarray:
    """Collapse the 80-gate circuit into W [1024, 1024] (float64),
    with out_row = x_row @ W."""
    c = cores.astype(np.float64)
    s = np.eye(D, dtype=np.float64).reshape((D,) + (BOND,) * Q)
    for layer in range(N_LAYERS):
        base = layer * Q
        for g, (i, j) in enumerate(PAIRS1):
            s = np.tensordot(s, c[base + g], axes=((i + 1, j + 1), (0, 1)))
            s = np.moveaxis(s, (-2, -1), (i + 1, j + 1))
        for g, (i, j) in enumerate(PAIRS2):
            s = np.tensordot(s, c[base + HALF + g], axes=((i + 1, j + 1), (0, 1)))
            s = np.moveaxis(s, (-2, -1), (i + 1, j + 1))
    return s.reshape(D, D)


_NC_CACHE = None


def _build_bass(inv_scale: float):
    """Device program (identical on all 8 cores). Layouts (p = partition):
      x8d  [128, JC*T8*512]  e4m3:  x8d[p, j*3072 + t*512 + n]  = x8[t*128+p, j*512+n]
      x16d [128, JC*T16*512] fp16:  x16d[p, j*1024 + t*512 + n] = x16[t*128+p, j*512+n]
      w8d  [128, T8*1024]    e4m3:  w8d[p, t*1024 + m]  = (W[sel8]*S)[t*128+p, m]
      w16d [128, T16*1024]   fp16:  w16d[p, t*1024 + m] = (W[sel16]*S)[t*128+p, m]
      ytd  [128, JC*MC*512]  fp16:  ytd[p, (j*4+mp)*1024 + h*512 + n]
                                      = y[j*512+n, mp*256 + h*128 + p] (pre 1/S fold)
    """
    global _NC_CACHE
    if _NC_CACHE is not None:
        return _NC_CACHE

    import concourse.bacc as bacc
    import concourse.mybir as mybir
    import concourse.tile as tile

    F8 = mybir.dt.float8e4
    F16 = mybir.dt.float16
    F32 = mybir.dt.float32
    DR = mybir.MatmulPerfMode.DoubleRow

    nc = bacc.Bacc("TRN2")
    x8d = nc.dram_tensor("x8d", [128, JC * T8 * 512], F8, kind="ExternalInput")
    x16d = nc.dram_tensor("x16d", [128, JC * T16 * 512], F16, kind="ExternalInput")
    w8d = nc.dram_tensor("w8d", [128, T8 * D], F8, kind="ExternalInput")
    w16d = nc.dram_tensor("w16d", [128, T16 * D], F16, kind="ExternalInput")
    ytd = nc.dram_tensor("ytd", [128, JC * MC * 512], F16, kind="ExternalOutput")

    with tile.TileContext(nc) as tc:
        with (
            tc.tile_pool(name="xpool", bufs=1) as xpool,
            tc.tile_pool(name="wpool", bufs=1) as wpool,
            tc.tile_pool(name="opool", bufs=2) as opool,
            tc.tile_pool(name="psum", bufs=1, space="PSUM") as ppool,
        ):
            # ---- PE warmup: matmuls on zeros cover the HAM clock ramp
            # while the first x8/w8 DMA pieces are in flight (~2.3us).
            warm = xpool.tile([128, 512], F16, name="warm", tag="warm")
            nc.gpsimd.memset(warm[:], 0)
            wps = ppool.tile([128, 512], F32, name="wps", tag="ps7")
            for _ in range(5):
                nc.tensor.matmul(wps[0:128, :], warm[:, :128], warm[:])

            # ---- weight loads on the Scalar ring: w8's first pair-piece
            # covers phase 1's first 8 matmuls, w16 is only needed at phase 2.
            w8t = wpool.tile([128, T8 * D], F8, name="w8t", tag="w8t")
            nc.scalar.dma_start(w8t[:, : 2 * D], w8d[:, : 2 * D])
            nc.scalar.dma_start(w8t[:, 2 * D :], w8d[:, 2 * D :])
            w16t = wpool.tile([128, T16 * D], F16, name="w16t", tag="w16t")
            nc.scalar.dma_start(w16t[:], w16d[:])

            # ---- x chunk loads. j=0 goes on the otherwise-idle Sync ring in
            # pieces (smallest first) so the first DoubleRow pair's data lands
            # ASAP; j=1..7 stream on the Scalar ring behind the w loads.
            x8j = []
            x16j = []
            for j in range(JC):
                t8 = xpool.tile([128, T8 * 512], F8, name=f"x8j{j}", tag=f"x8{j}")
                src8 = x8d[:, j * T8 * 512 : (j + 1) * T8 * 512]
                if j == 0:
                    nc.sync.dma_start(t8[:, : 2 * 512], src8[:, : 2 * 512])
                    nc.sync.dma_start(t8[:, 2 * 512 :], src8[:, 2 * 512 :])
                else:
                    nc.scalar.dma_start(t8[:], src8)
                x8j.append(t8)
                t16 = xpool.tile([128, T16 * 512], F16, name=f"x16j{j}", tag=f"x16{j}")
                eng = nc.sync if j == 0 else nc.scalar
                eng.dma_start(t16[:], x16d[:, j * T16 * 512 : (j + 1) * T16 * 512])
                x16j.append(t16)

            # ---- main loop over batch chunks
            def dr_mm(psum, m, tp, j, start):
                x8v = x8j[j][:].rearrange("p (t n) -> p t n", n=512)
                w8v = w8t[:].rearrange("p (t m) -> p t m", m=D)
                nc.tensor.matmul(
                    psum[:],
                    w8v[:, 2 * tp : 2 * tp + 2, m * 128 : (m + 1) * 128],
                    x8v[:, 2 * tp : 2 * tp + 2, :],
                    start=start,
                    stop=False,
                    perf_mode=DR,
                )

            def f16_mm(psum, m, t, j):
                nc.tensor.matmul(
                    psum[:],
                    w16t[:, t * D + m * 128 : t * D + (m + 1) * 128],
                    x16j[j][:, t * 512 : (t + 1) * 512],
                    start=False,
                    stop=(t == T16 - 1),
                )

            osb_live = [None]

            def drain(psums, m, j, last_j):
                mp = m // 2
                out_off = (j * (MC // 2) + mp) * 1024
                if m % 2 == 0:
                    osb = opool.tile(
                        [128, 2 * 512], F16, name=f"osb{mp}", tag=f"osb{mp}"
                    )
                    osb_live[0] = osb
                    nc.vector.tensor_scalar_mul(osb[:, :512], psums[m][:], inv_scale)
                    if last_j and mp == MC // 2 - 1:
                        # fire m6's half early; the tail then only waits on m7
                        nc.sync.dma_start(ytd[:, out_off : out_off + 512], osb[:, :512])
                    return
                osb = osb_live[0]
                if last_j and mp == MC // 2 - 1:
                    # split the final drain across both engines, then one
                    # small 128KB transfer closes the kernel
                    nc.scalar.mul(osb[:, 512:768], psums[m][:, :256], inv_scale)
                    nc.vector.tensor_scalar_mul(
                        osb[:, 768:], psums[m][:, 256:], inv_scale
                    )
                    nc.sync.dma_start(ytd[:, out_off + 512 : out_off + 1024], osb[:, 512:])
                else:
                    nc.scalar.mul(osb[:, 512:], psums[m][:], inv_scale)
                    nc.sync.dma_start(ytd[:, out_off : out_off + 1024], osb[:])

            for j in range(JC):
                psums = [
                    ppool.tile([128, 512], F32, name=f"ps{m}", tag=f"ps{m}")
                    for m in range(MC)
                ]
                last_j = j == JC - 1
                if j == 0:
                    # k-pair-outer: first w8/x8 pieces cover the first 8 MMs
                    for tp in range(T8 // 2):
                        for m in range(MC):
                            dr_mm(psums[m], m, tp, j, start=(tp == 0))
                    for m in range(MC):
                        for t in range(T16):
                            f16_mm(psums[m], m, t, j)
                        drain(psums, m, j, last_j)
                else:
                    # m-outer full-finish: drains + output DMA chase compute
                    for m in range(MC):
                        for tp in range(T8 // 2):
                            dr_mm(psums[m], m, tp, j, start=(tp == 0))
                        for t in range(T16):
                            f16_mm(psums[m], m, t, j)
                        drain(psums, m, j, last_j)

    nc.compile()
    _NC_CACHE = nc
    return nc


def _prepare(x: np.ndarray, cores: np.ndarray):
    """Host-side: build W, pick the fp8/fp16 row split, quantize and pack
    operands into the per-partition-contiguous device layouts."""
    W = build_w(cores)
    rn = np.sqrt((W * W).sum(axis=1))
    order = np.argsort(rn, kind="stable")
    sel8 = order[:K8]
    sel16 = order[K8:]

    amax8 = float(np.abs(W[sel8]).max())
    amax16 = float(np.abs(W[sel16]).max())
    # keep W8 under e4m3's 240 max-normal and W16*S comfortably inside fp16
    s_pow = min(
        math.floor(math.log2(216.0 / max(amax8, 1e-30))),
        math.floor(math.log2(30000.0 / max(amax16, 1e-30))),
    )
    S = float(2.0**s_pow)

    # w8d[p, t*D + m] = (W[sel8]*S)[t*128+p, m]
    w8d = np.ascontiguousarray(
        (W[sel8] * S)
        .astype(np.float32)
        .astype(ml_dtypes.float8_e4m3)
        .reshape(T8, 128, D)
        .transpose(1, 0, 2)
        .reshape(128, T8 * D)
    )
    w16d = np.ascontiguousarray(
        (W[sel16] * S)
        .astype(np.float32)
        .astype(np.float16)
        .reshape(T16, 128, D)
        .transpose(1, 0, 2)
        .reshape(128, T16 * D)
    )

    # x8 [K8, B] then per-core pack to [128, JC, T8, 512]
    x8_full = x[:, sel8].astype(np.float32).astype(ml_dtypes.float8_e4m3).T
    x16_full = x[:, sel16].astype(np.float32).astype(np.float16).T
    return w8d, w16d, x8_full, x16_full, 1.0 / S


def _pack_x(xf: np.ndarray, c: int, tcount: int):
    """[tcount*128, B] core shard -> [128, JC*tcount*512] device layout."""
    shard = xf[:, c * NPC : (c + 1) * NPC]
    return np.ascontiguousarray(
        shard.reshape(tcount, 128, JC, 512)
        .transpose(1, 2, 0, 3)
        .reshape(128, JC * tcount * 512)
    )


def _run(x: np.ndarray, cores: np.ndarray, trace: bool = False, trace_cores=None):
    from concourse.bass_utils import run_bass_kernel_spmd

    w8d, w16d, x8_full, x16_full, inv_scale = _prepare(x, cores)

    in_maps = []
    for c in range(N_CORES):
        in_maps.append(
            {
                "x8d": _pack_x(x8_full, c, T8),
                "x16d": _pack_x(x16_full, c, T16),
                "w8d": w8d,
                "w16d": w16d,
            }
        )

    nc = _build_bass(inv_scale)
    kwargs = {}
    if trace_cores is not None:
        kwargs["trace_cores"] = trace_cores
    res = run_bass_kernel_spmd(
        nc, in_maps, core_ids=list(range(N_CORES)), trace=trace, **kwargs
    )

    y = np.empty((B, D), dtype=np.float32)
    for c in range(N_CORES):
        # ytd[p, j, mp, h, n] = y[j*512+n, mp*256+h*128+p]
        arr = res.results[c]["ytd"].reshape(128, JC, MC // 2, 2, 512)
        y[c * NPC : (c + 1) * NPC, :] = (
            arr.transpose(1, 4, 2, 3, 0).reshape(NPC, D).astype(np.float32)
        )
    return y, res


def kernel(x: np.ndarray, cores: np.ndarray) -> np.ndarray:
    y, _ = _run(x, cores, trace=False)
    return y
